# revision 32
# baseline (speedup 1.0000x reference)
"""Bidirectional ConvLSTM block for Trainium2 (Bass/Tile), 8-core SPMD.

Problem: x [S=16, B=4, Cin=32, H=128, W=128] f32, Wf/Wb [128, 64, 3, 3],
bf/bb [128].  Output [S, B, 2*Co=64, H, W]: forward ConvLSTM hidden states
concat backward ConvLSTM (run on time-reversed x, not re-flipped).

Sharding: 8 independent recurrences = 2 directions x 4 batch elements.
Core k runs direction d=k//4 on batch b=k%4.  No cross-core communication.

Per-core kernel design (v2):
  - SBUF "act" tile [128 part, 130*130] bf16 (ping/pong): zero-padded
    (H+2)x(W+2) spatial plane per channel.
      partitions  0-31 : x_t   (center copy)
      partitions 32-63 : h_{t-1} (center copy)
      partitions 64-95 : x_t   shifted right by one column
      partitions 96-127: h_{t-1} shifted
    3x3 conv => 6 matmul passes per gate: 3 passes pair (dy,0)+(dy,-1)
    via the shifted rows; 3 passes do (dy,+1) with zero weights on them.
  - Per group (16 image rows = 4 spatial tiles x 512 positions): one PSUM
    tile [128, 2048] (4 banks), gate g in cols 512g.  Col-tiled matmuls
    (tile_position (0,32j)) write [32j:32j+32, 512g:...]: partitions =
    32*tile + channel, so pointwise runs on full 128-partition tiles.
    psum pool bufs=2 -> groups double-buffer 4+4 banks, PE never waits.
  - All dma_starts ride the Sync queue (plus x loads); the Scalar queue
    carries ONLY activations (v1 put h-scatter DMAs there, each DIRECT2D
    blocking the act FIFO ~0.7-6.5us -> PE starve -> HAM re-throttle).
  - y stored bf16 in native SBUF order [S, group, 128, 512] with one
    contiguous DMA per group; host reassembles/upcasts.
  - h written once (bf16): y DMA + 2 merged plane scatters per group.
"""

import os
import sys

import numpy as np

for _p in ("/opt/trn_rl_repo", "/root/.axon_site/_ro/trn_rl_repo"):
    if os.path.isdir(_p) and _p not in sys.path:
        sys.path.insert(0, _p)

import ml_dtypes  # noqa: E402
import concourse.bass as bass  # noqa: E402,F401
import concourse.mybir as mybir  # noqa: E402
from concourse import bacc, tile  # noqa: E402
from concourse.bass_utils import run_bass_kernel_spmd  # noqa: E402

F32 = mybir.dt.float32
BF16 = mybir.dt.bfloat16
AF = mybir.ActivationFunctionType

S, B, CIN, H, W = 16, 4, 32, 128, 128
CO = 32
HP, WP = H + 2, W + 2          # 130 x 130 padded plane
PADN = HP * WP                 # 16900
NSP = H * W                    # 16384
NT = 512                       # spatial positions per matmul tile (4 rows)
TPG = 4                        # tiles per group (col-tiled together)
GROUPS = NSP // (NT * TPG)     # 8 groups per step; group = 16 image rows
N_CORES = 8


def build_kernel(nc, tc, x_ap, w_ap, b_ap, y_ap, p0_ap, n_steps, zero_bias):
    ctx_pools = []

    def pool(**kw):
        p = tc.tile_pool(**kw)
        ctx_pools.append(p)
        return p.__enter__()

    const = pool(name="const", bufs=1)
    tmp = pool(name="tmp", bufs=3)
    psum = pool(name="psum", bufs=2, space="PSUM")

    # Persistent tiles
    a0 = const.tile([128, PADN], BF16, tag="act0")
    a1 = const.tile([128, PADN], BF16, tag="act1")
    acts = [a0, a1]
    ctile = const.tile([128, GROUPS * NT], F32, tag="c")
    wsb = const.tile([128, 18 * 32], BF16, tag="w")
    bsb = const.tile([128, 4], F32, tag="bias")

    # Step-0 act plane comes fully host-built (x copies + zeroed h + pads),
    # loaded in row chunks so group 0's matmuls start after ~19 rows land.
    nc.sync.dma_start(wsb[:, :], w_ap)
    nc.sync.dma_start(a0[:, 0 : 19 * WP], p0_ap[:, 0 : 19 * WP])
    for lo, hi in ((19, 51), (51, 83), (83, HP)):
        nc.sync.dma_start(a0[:, lo * WP : hi * WP], p0_ap[:, lo * WP : hi * WP])
    nc.sync.dma_start(bsb[:, :], b_ap)

    # --- one-time zero init (a1 h-region borders only; its x regions are
    # re-loaded host-padded every step, h interiors are scatter-written) ---
    ar1 = a1.rearrange("p (r w) -> p r w", r=HP)
    nc.gpsimd.memset(a1[32:64, 0:WP], 0.0)                  # pad row 0
    nc.gpsimd.memset(a1[96:128, 0:WP], 0.0)
    nc.gpsimd.memset(a1[32:64, (HP - 1) * WP :], 0.0)       # pad row 129
    nc.gpsimd.memset(a1[96:128, (HP - 1) * WP :], 0.0)
    nc.gpsimd.memset(ar1[32:64, :, 0:1], 0.0)               # col 0 (unread, keep finite)
    nc.gpsimd.memset(ar1[32:64, :, WP - 1 : WP], 0.0)       # center col 129
    nc.gpsimd.memset(ar1[96:128, :, 0:2], 0.0)              # shift cols 0-1
    nc.vector.memset(ctile[:, :], 0.0)

    def load_x(t):
        # x arrives host-padded to the full 130x130 plane: both the center
        # copy and the +1-shifted copy are single contiguous runs per
        # partition (the shift picks up its zero border from the host pad).
        nc.sync.dma_start(acts[t % 2][0:32, :], x_ap[t])
        nc.sync.dma_start(acts[t % 2][64:96, 1:PADN], x_ap[t, :, 0 : PADN - 1])

    from concourse.ap import AP as _AP

    def scatter_ap(a_nxt, pbase, q, j, coloff):
        """3D dst AP: partitions pbase..pbase+32, rows 64q+j+1 + 4k (k=0..16),
        cols coloff..coloff+128 of the 130x130 plane.  (Spatial tile j holds
        rows == j mod 4 of its quad, so the 16 staged rows form one stride-4
        run.)"""
        base = a_nxt[:, :]
        base_row = 64 * q + j + 1
        off = pbase * PADN + base_row * WP + coloff
        return _AP(
            base.tensor,
            base.offset + off,
            [[PADN, 32], [4 * WP, 16], [1, W]],
        )

    for t in range(n_steps):
        a_cur = acts[t % 2]
        a_nxt = acts[(t + 1) % 2]
        ar_cur = a_cur.rearrange("p (r w) -> p r w", r=HP)
        if t + 1 < n_steps:
            load_x(t + 1)

        hq = None
        for grp in range(GROUPS):
            if grp % 4 == 0:
                hq = tmp.tile(
                    [128, 4 * NT], BF16, tag="hq", name=f"hq{t}_{grp // 4}"
                )
            # Hybrid pass schedule, 18 slot-times per group (vs 24 all-full):
            # gate 2p: 3 full K=128 passes pairing (dy,0)+(dy,-1);
            # gate 2p+1: 3 full passes at col offset +1 pairing (dy,+1)+(dy,0);
            # leftovers -- 2p's (dy,+1) and 2p+1's (dy,-1) -- share 3 row-
            # split K=64 slots (center half / shifted half run concurrently
            # into their own PSUM banks via row tiling).
            zt = psum.tile([128, 4 * NT], F32, tag="z", name=f"z{t}_{grp}")
            for pg in range(2):
                ga, gb = 2 * pg, 2 * pg + 1
                # split slots first (start=True): banks then complete during
                # the full passes, releasing acts earlier
                for p3 in range(3):
                    dy = p3 - 1
                    blk = (pg * 9 + 6 + p3) * 32
                    for j in range(TPG):
                        r0 = 16 * grp + j + 1 + dy
                        rows = slice(r0, r0 + 13, 4)
                        nc.tensor.matmul(
                            zt[32 * j : 32 * j + 32, ga * NT : (ga + 1) * NT],
                            wsb[0:64, blk : blk + 32],
                            ar_cur[0:64, rows, 2 : W + 2],
                            start=(p3 == 0),
                            stop=False,
                            skip_group_check=True,
                            tile_position=(0, 32 * j),
                        )
                        nc.tensor.matmul(
                            zt[32 * j : 32 * j + 32, gb * NT : (gb + 1) * NT],
                            wsb[64:128, blk : blk + 32],
                            ar_cur[64:128, rows, 1 : W + 1],
                            start=(p3 == 0),
                            stop=False,
                            skip_group_check=True,
                            tile_position=(64, 32 * j),
                        )
                for half, g in ((0, ga), (1, gb)):
                    for p3 in range(3):
                        dy = p3 - 1
                        blk = (pg * 9 + half * 3 + p3) * 32
                        for j in range(TPG):
                            r0 = 16 * grp + j + 1 + dy
                            nc.tensor.matmul(
                                zt[32 * j : 32 * j + 32, g * NT : (g + 1) * NT],
                                wsb[:, blk : blk + 32],
                                ar_cur[:, r0 : r0 + 13 : 4, 1 + half : W + 1 + half],
                                start=False,
                                stop=(p3 == 2),
                                skip_group_check=True,
                                tile_position=(0, 32 * j),
                            )

            # ---- pointwise ----
            csl = ctile[:, grp * NT : (grp + 1) * NT]
            sio = tmp.tile([128, 3 * NT], BF16, tag="sio", name=f"sio{t}_{grp}")
            tg = tmp.tile([128, NT], BF16, tag="tg", name=f"tg{t}_{grp}")
            if zero_bias:
                nc.scalar.activation(sio[:, :], zt[:, 0 : 3 * NT], AF.Sigmoid)
                nc.scalar.activation(tg[:, :], zt[:, 3 * NT :], AF.Tanh)
            else:
                for g, sl in ((0, 0), (1, 1), (2, 2)):
                    nc.scalar.activation(
                        sio[:, sl * NT : (sl + 1) * NT],
                        zt[:, g * NT : (g + 1) * NT],
                        AF.Sigmoid,
                        bias=bsb[:, g : g + 1],
                    )
                nc.scalar.activation(
                    tg[:, :], zt[:, 3 * NT :], AF.Tanh, bias=bsb[:, 3:4]
                )
            si = sio[:, 0:NT]
            sf = sio[:, NT : 2 * NT]
            so = sio[:, 2 * NT : 3 * NT]

            t2 = tmp.tile([128, NT], F32, tag="t2", name=f"t2_{t}_{grp}")
            t3 = tmp.tile([128, NT], F32, tag="t3", name=f"t3_{t}_{grp}")
            nc.vector.tensor_mul(t3[:, :], sf, csl)
            nc.vector.tensor_mul(t2[:, :], si, tg[:, :])
            nc.vector.tensor_add(csl, t2[:, :], t3[:, :])

            tcn = tmp.tile([128, NT], BF16, tag="tcn", name=f"tcn{t}_{grp}")
            nc.scalar.activation(tcn[:, :], csl, AF.Tanh)
            gq = grp % 4
            hsl = hq[:, gq * NT : (gq + 1) * NT]
            nc.vector.tensor_mul(hsl, so, tcn[:, :])

            if t == n_steps - 1:
                # last step: per-group y so the final DMA drains during the
                # trailing pointwise chain instead of after it
                nc.sync.dma_start(y_ap[t, :, grp * NT : (grp + 1) * NT], hsl)
            if grp % 4 == 3:
                q = grp // 4
                if t < n_steps - 1:
                    # y out: one contiguous DMA per quad, SBUF-native layout
                    nc.sync.dma_start(
                        y_ap[t, :, q * 4 * NT : (q + 1) * 4 * NT], hq[:, :]
                    )
                if t + 1 < n_steps:
                    for j in range(TPG):
                        src = hq[32 * j : 32 * j + 32, :]
                        nc.sync.dma_start(scatter_ap(a_nxt, 32, q, j, 1), src)
                        nc.sync.dma_start(scatter_ap(a_nxt, 96, q, j, 2), src)

    for p in reversed(ctx_pools):
        p.__exit__(None, None, None)


def build_program(n_steps=S, zero_bias=True):
    nc = bacc.Bacc(
        "TRN2",
        target_bir_lowering=False,
        debug=False,
        enable_asserts=False,
        num_devices=N_CORES,
    )
    x_d = nc.dram_tensor("x", [n_steps, CIN, PADN], BF16, kind="ExternalInput")
    w_d = nc.dram_tensor("w", [128, 18 * 32], BF16, kind="ExternalInput")
    b_d = nc.dram_tensor("bias", [128, 4], F32, kind="ExternalInput")
    y_d = nc.dram_tensor("y", [n_steps, 128, GROUPS * NT], BF16, kind="ExternalOutput")
    p0_d = nc.dram_tensor("plane0", [128, PADN], BF16, kind="ExternalInput")
    with tile.TileContext(nc) as tc:
        build_kernel(
            nc, tc, x_d.ap(), w_d.ap(), b_d.ap(), y_d.ap(), p0_d.ap(),
            n_steps, zero_bias,
        )
    nc.compile()
    return nc


def pack_weights(Wd):
    """Wd [128, 64, 3, 3] f32 -> lhsT blocks [128, 18*32] bf16.
    Per gate pair: 3 full blocks gate a (center kx=1, shift kx=0), 3 full
    blocks gate b (center kx=2, shift kx=1), 3 split blocks (rows 0-63 =
    gate a kx=2 via center; rows 64-127 = gate b kx=0 via shift)."""
    wp = np.zeros((128, 18, 32), np.float32)
    for pg in range(2):
        Wa = Wd[(2 * pg) * 32 : (2 * pg + 1) * 32]      # [32(m), 64, 3, 3]
        Wb_ = Wd[(2 * pg + 1) * 32 : (2 * pg + 2) * 32]
        for ky in range(3):
            blk = wp[:, pg * 9 + ky, :]
            blk[0:32, :] = Wa[:, 0:32, ky, 1].T
            blk[32:64, :] = Wa[:, 32:64, ky, 1].T
            blk[64:96, :] = Wa[:, 0:32, ky, 0].T
            blk[96:128, :] = Wa[:, 32:64, ky, 0].T
            blk = wp[:, pg * 9 + 3 + ky, :]
            blk[0:32, :] = Wb_[:, 0:32, ky, 2].T
            blk[32:64, :] = Wb_[:, 32:64, ky, 2].T
            blk[64:96, :] = Wb_[:, 0:32, ky, 1].T
            blk[96:128, :] = Wb_[:, 32:64, ky, 1].T
            blk = wp[:, pg * 9 + 6 + ky, :]
            blk[0:32, :] = Wa[:, 0:32, ky, 2].T
            blk[32:64, :] = Wa[:, 32:64, ky, 2].T
            blk[64:96, :] = Wb_[:, 0:32, ky, 0].T
            blk[96:128, :] = Wb_[:, 32:64, ky, 0].T
    return wp.reshape(128, 18 * 32).astype(ml_dtypes.bfloat16)


def pack_bias(bd):
    """bd [128] f32 -> [128, 4] f32 (partition p = 32*tile + ch)."""
    bp = np.zeros((128, 4), np.float32)
    for g in range(4):
        bp[:, g] = np.tile(bd[g * 32 : (g + 1) * 32], 4)
    return bp


def make_in_maps(x, Wf, bf, Wb, bb, n_steps=S):
    wpacks = [pack_weights(np.asarray(Wf, np.float32)),
              pack_weights(np.asarray(Wb, np.float32))]
    bpacks = [pack_bias(np.asarray(bf, np.float32)),
              pack_bias(np.asarray(bb, np.float32))]
    x = np.asarray(x, np.float32)
    in_maps = []
    for k in range(N_CORES):
        d, b = k // 4, k % 4
        xc = x[:n_steps, b] if d == 0 else x[::-1][:n_steps, b]
        xp = np.zeros((n_steps, CIN, HP, WP), ml_dtypes.bfloat16)
        xp[:, :, 1 : H + 1, 1 : W + 1] = xc
        p0 = np.zeros((128, HP, WP), ml_dtypes.bfloat16)
        p0[0:32] = xp[0]
        p0[64:96, :, 1:] = xp[0, :, :, :-1]
        in_maps.append(
            {
                "x": xp.reshape(n_steps, CIN, PADN),
                "w": wpacks[d],
                "bias": bpacks[d],
                "plane0": p0.reshape(128, PADN),
            }
        )
    return in_maps


_CACHED_NC = {}


def kernel(x, Wf, bf, Wb, bb):
    zero_bias = (not np.any(np.asarray(bf))) and (not np.any(np.asarray(bb)))
    nc = _CACHED_NC.get(zero_bias)
    if nc is None:
        nc = _CACHED_NC[zero_bias] = build_program(S, zero_bias)
    in_maps = make_in_maps(x, Wf, bf, Wb, bb)
    res = run_bass_kernel_spmd(nc, in_maps, core_ids=list(range(N_CORES)))
    out = np.empty((S, B, 2 * CO, H, W), np.float32)
    for k in range(N_CORES):
        d, b = k // 4, k % 4
        yk = np.asarray(res.results[k]["y"], dtype=np.float32)
        # [S, (j c), (q g r w)] -> [S, c, q, g, r, j, w]; row = 64q+16g+4r+j
        yk = yk.reshape(S, TPG, CO, 2, 4, 4, W).transpose(0, 2, 3, 4, 5, 1, 6)
        out[:, b, d * CO : (d + 1) * CO] = yk.reshape(S, CO, H, W)
    return out


if __name__ == "__main__":
    import jax

    jax.config.update("jax_platforms", "cpu")
    rng = np.random.default_rng(0)
    x = rng.standard_normal((S, B, CIN, H, W), np.float32)
    Wf = (rng.standard_normal((128, 64, 3, 3)) * 0.05).astype(np.float32)
    Wb = (rng.standard_normal((128, 64, 3, 3)) * 0.05).astype(np.float32)
    bf = np.zeros(128, np.float32)
    bb = np.zeros(128, np.float32)
    y = kernel(x, Wf, bf, Wb, bb)
    print("out", y.shape, y.dtype)


# revision 33
# speedup vs baseline: 1.1830x; 1.1830x over previous
"""Bidirectional ConvLSTM block for Trainium2 (Bass/Tile), 8-core SPMD.

Problem: x [S=16, B=4, Cin=32, H=128, W=128] f32, Wf/Wb [128, 64, 3, 3],
bf/bb [128].  Output [S, B, 2*Co=64, H, W]: forward ConvLSTM hidden states
concat backward ConvLSTM (run on time-reversed x, not re-flipped).

Sharding: 8 independent recurrences = 2 directions x 4 batch elements.
Core k runs direction d=k//4 on batch b=k%4.  No cross-core communication.

Per-core kernel design (v2):
  - SBUF "act" tile [128 part, 130*130] bf16 (ping/pong): zero-padded
    (H+2)x(W+2) spatial plane per channel.
      partitions  0-31 : x_t   (center copy)
      partitions 32-63 : h_{t-1} (center copy)
      partitions 64-95 : x_t   shifted right by one column
      partitions 96-127: h_{t-1} shifted
    3x3 conv => 6 matmul passes per gate: 3 passes pair (dy,0)+(dy,-1)
    via the shifted rows; 3 passes do (dy,+1) with zero weights on them.
  - Per group (16 image rows = 4 spatial tiles x 512 positions): one PSUM
    tile [128, 2048] (4 banks), gate g in cols 512g.  Col-tiled matmuls
    (tile_position (0,32j)) write [32j:32j+32, 512g:...]: partitions =
    32*tile + channel, so pointwise runs on full 128-partition tiles.
    psum pool bufs=2 -> groups double-buffer 4+4 banks, PE never waits.
  - All dma_starts ride the Sync queue (plus x loads); the Scalar queue
    carries ONLY activations (v1 put h-scatter DMAs there, each DIRECT2D
    blocking the act FIFO ~0.7-6.5us -> PE starve -> HAM re-throttle).
  - y stored bf16 in native SBUF order [S, group, 128, 512] with one
    contiguous DMA per group; host reassembles/upcasts.
  - h written once (bf16): y DMA + 2 merged plane scatters per group.
"""

import os
import sys

import numpy as np

for _p in ("/opt/trn_rl_repo", "/root/.axon_site/_ro/trn_rl_repo"):
    if os.path.isdir(_p) and _p not in sys.path:
        sys.path.insert(0, _p)

import ml_dtypes  # noqa: E402
import concourse.bass as bass  # noqa: E402,F401
import concourse.mybir as mybir  # noqa: E402
from concourse import bacc, tile  # noqa: E402
from concourse.bass_utils import run_bass_kernel_spmd  # noqa: E402

F32 = mybir.dt.float32
BF16 = mybir.dt.bfloat16
AF = mybir.ActivationFunctionType

S, B, CIN, H, W = 16, 4, 32, 128, 128
CO = 32
HP, WP = H + 2, W + 2          # 130 x 130 padded plane
PADN = HP * WP                 # 16900
NSP = H * W                    # 16384
NT = 512                       # spatial positions per matmul tile (4 rows)
TPG = 4                        # tiles per group (col-tiled together)
GROUPS = NSP // (NT * TPG)     # 8 groups per step; group = 16 image rows
N_CORES = 8


def build_kernel(nc, tc, x_ap, w_ap, b_ap, y_ap, p0_ap, n_steps, zero_bias):
    ctx_pools = []

    def pool(**kw):
        p = tc.tile_pool(**kw)
        ctx_pools.append(p)
        return p.__enter__()

    const = pool(name="const", bufs=1)
    tmp = pool(name="tmp", bufs=3)
    psum = pool(name="psum", bufs=2, space="PSUM")

    # Persistent tiles
    a0 = const.tile([128, PADN], BF16, tag="act0")
    a1 = const.tile([128, PADN], BF16, tag="act1")
    acts = [a0, a1]
    ctile = const.tile([128, GROUPS * NT], F32, tag="c")
    wsb = const.tile([128, 18 * 32], BF16, tag="w")
    bsb = const.tile([128, 4], F32, tag="bias")

    # Step-0 act plane comes fully host-built (x copies + zeroed h + pads),
    # loaded in row chunks so group 0's matmuls start after ~19 rows land.
    nc.sync.dma_start(wsb[:, :], w_ap)
    nc.sync.dma_start(a0[:, 0 : 19 * WP], p0_ap[:, 0 : 19 * WP])
    for lo, hi in ((19, 51), (51, 83), (83, HP)):
        nc.sync.dma_start(a0[:, lo * WP : hi * WP], p0_ap[:, lo * WP : hi * WP])
    nc.sync.dma_start(bsb[:, :], b_ap)

    # --- one-time zero init (a1 h-region borders only; its x regions are
    # re-loaded host-padded every step, h interiors are scatter-written) ---
    ar1 = a1.rearrange("p (r w) -> p r w", r=HP)
    nc.gpsimd.memset(a1[32:64, 0:WP], 0.0)                  # pad row 0
    nc.gpsimd.memset(a1[96:128, 0:WP], 0.0)
    nc.gpsimd.memset(a1[32:64, (HP - 1) * WP :], 0.0)       # pad row 129
    nc.gpsimd.memset(a1[96:128, (HP - 1) * WP :], 0.0)
    nc.gpsimd.memset(ar1[32:64, :, 0:1], 0.0)               # col 0 (unread, keep finite)
    nc.gpsimd.memset(ar1[32:64, :, WP - 1 : WP], 0.0)       # center col 129
    nc.gpsimd.memset(ar1[96:128, :, 0:2], 0.0)              # shift cols 0-1
    nc.vector.memset(ctile[:, :], 0.0)

    def load_x(t):
        # x arrives host-padded to the full 130x130 plane: both the center
        # copy and the +1-shifted copy are single contiguous runs per
        # partition (the shift picks up its zero border from the host pad).
        nc.sync.dma_start(acts[t % 2][0:32, :], x_ap[t])
        nc.sync.dma_start(acts[t % 2][64:96, 1:PADN], x_ap[t, :, 0 : PADN - 1])

    from concourse.ap import AP as _AP

    def scatter_ap(a_nxt, pbase, q, j, coloff):
        """3D dst AP: partitions pbase..pbase+32, rows 64q+j+1 + 4k (k=0..16),
        cols coloff..coloff+128 of the 130x130 plane.  (Spatial tile j holds
        rows == j mod 4 of its quad, so the 16 staged rows form one stride-4
        run.)"""
        base = a_nxt[:, :]
        base_row = 64 * q + j + 1
        off = pbase * PADN + base_row * WP + coloff
        return _AP(
            base.tensor,
            base.offset + off,
            [[PADN, 32], [4 * WP, 16], [1, W]],
        )

    for t in range(n_steps):
        a_cur = acts[t % 2]
        a_nxt = acts[(t + 1) % 2]
        ar_cur = a_cur.rearrange("p (r w) -> p r w", r=HP)
        if t + 1 < n_steps:
            load_x(t + 1)

        hq = None
        for grp in range(GROUPS):
            if grp % 4 == 0:
                hq = tmp.tile(
                    [128, 4 * NT], BF16, tag="hq", name=f"hq{t}_{grp // 4}"
                )
            # Hybrid pass schedule, 18 slot-times per group (vs 24 all-full):
            # gate 2p: 3 full K=128 passes pairing (dy,0)+(dy,-1);
            # gate 2p+1: 3 full passes at col offset +1 pairing (dy,+1)+(dy,0);
            # leftovers -- 2p's (dy,+1) and 2p+1's (dy,-1) -- share 3 row-
            # split K=64 slots (center half / shifted half run concurrently
            # into their own PSUM banks via row tiling).
            zt = psum.tile([128, 4 * NT], F32, tag="z", name=f"z{t}_{grp}")
            for pg in range(2):
                ga, gb = 2 * pg, 2 * pg + 1
                for half, g in ((0, ga), (1, gb)):
                    for p3 in range(3):
                        dy = p3 - 1
                        blk = (pg * 9 + half * 3 + p3) * 32
                        for j in range(TPG):
                            r0 = 16 * grp + j + 1 + dy
                            nc.tensor.matmul(
                                zt[32 * j : 32 * j + 32, g * NT : (g + 1) * NT],
                                wsb[:, blk : blk + 32],
                                ar_cur[:, r0 : r0 + 13 : 4, 1 + half : W + 1 + half],
                                start=(p3 == 0),
                                stop=False,
                                skip_group_check=True,
                                tile_position=(0, 32 * j),
                            )
                for p3 in range(3):
                    dy = p3 - 1
                    blk = (pg * 9 + 6 + p3) * 32
                    for j in range(TPG):
                        r0 = 16 * grp + j + 1 + dy
                        rows = slice(r0, r0 + 13, 4)
                        nc.tensor.matmul(
                            zt[32 * j : 32 * j + 32, ga * NT : (ga + 1) * NT],
                            wsb[0:64, blk : blk + 32],
                            ar_cur[0:64, rows, 2 : W + 2],
                            start=False,
                            stop=(p3 == 2),
                            skip_group_check=True,
                            tile_position=(0, 32 * j),
                        )
                        nc.tensor.matmul(
                            zt[32 * j : 32 * j + 32, gb * NT : (gb + 1) * NT],
                            wsb[64:128, blk : blk + 32],
                            ar_cur[64:128, rows, 1 : W + 1],
                            start=False,
                            stop=(p3 == 2),
                            skip_group_check=True,
                            tile_position=(64, 32 * j),
                        )

            # ---- pointwise ----
            csl = ctile[:, grp * NT : (grp + 1) * NT]
            sio = tmp.tile([128, 3 * NT], BF16, tag="sio", name=f"sio{t}_{grp}")
            tg = tmp.tile([128, NT], BF16, tag="tg", name=f"tg{t}_{grp}")
            if zero_bias:
                nc.scalar.activation(sio[:, :], zt[:, 0 : 3 * NT], AF.Sigmoid)
                nc.scalar.activation(tg[:, :], zt[:, 3 * NT :], AF.Tanh)
            else:
                for g, sl in ((0, 0), (1, 1), (2, 2)):
                    nc.scalar.activation(
                        sio[:, sl * NT : (sl + 1) * NT],
                        zt[:, g * NT : (g + 1) * NT],
                        AF.Sigmoid,
                        bias=bsb[:, g : g + 1],
                    )
                nc.scalar.activation(
                    tg[:, :], zt[:, 3 * NT :], AF.Tanh, bias=bsb[:, 3:4]
                )
            si = sio[:, 0:NT]
            sf = sio[:, NT : 2 * NT]
            so = sio[:, 2 * NT : 3 * NT]

            t2 = tmp.tile([128, NT], F32, tag="t2", name=f"t2_{t}_{grp}")
            t3 = tmp.tile([128, NT], F32, tag="t3", name=f"t3_{t}_{grp}")
            nc.vector.tensor_mul(t3[:, :], sf, csl)
            nc.vector.tensor_mul(t2[:, :], si, tg[:, :])
            nc.vector.tensor_add(csl, t2[:, :], t3[:, :])

            tcn = tmp.tile([128, NT], BF16, tag="tcn", name=f"tcn{t}_{grp}")
            nc.scalar.activation(tcn[:, :], csl, AF.Tanh)
            gq = grp % 4
            hsl = hq[:, gq * NT : (gq + 1) * NT]
            nc.vector.tensor_mul(hsl, so, tcn[:, :])

            if t == n_steps - 1:
                # last step: per-group y so the final DMA drains during the
                # trailing pointwise chain instead of after it
                nc.sync.dma_start(y_ap[t, :, grp * NT : (grp + 1) * NT], hsl)
            if grp % 4 == 3:
                q = grp // 4
                if t < n_steps - 1:
                    # y out: one contiguous DMA per quad, SBUF-native layout
                    nc.sync.dma_start(
                        y_ap[t, :, q * 4 * NT : (q + 1) * 4 * NT], hq[:, :]
                    )
                if t + 1 < n_steps:
                    for j in range(TPG):
                        src = hq[32 * j : 32 * j + 32, :]
                        nc.sync.dma_start(scatter_ap(a_nxt, 32, q, j, 1), src)
                        nc.sync.dma_start(scatter_ap(a_nxt, 96, q, j, 2), src)

    for p in reversed(ctx_pools):
        p.__exit__(None, None, None)


def build_program(n_steps=S, zero_bias=True):
    nc = bacc.Bacc(
        "TRN2",
        target_bir_lowering=False,
        debug=False,
        enable_asserts=False,
        num_devices=N_CORES,
    )
    x_d = nc.dram_tensor("x", [n_steps, CIN, PADN], BF16, kind="ExternalInput")
    w_d = nc.dram_tensor("w", [128, 18 * 32], BF16, kind="ExternalInput")
    b_d = nc.dram_tensor("bias", [128, 4], F32, kind="ExternalInput")
    y_d = nc.dram_tensor("y", [n_steps, 128, GROUPS * NT], BF16, kind="ExternalOutput")
    p0_d = nc.dram_tensor("plane0", [128, PADN], BF16, kind="ExternalInput")
    with tile.TileContext(nc) as tc:
        build_kernel(
            nc, tc, x_d.ap(), w_d.ap(), b_d.ap(), y_d.ap(), p0_d.ap(),
            n_steps, zero_bias,
        )
    nc.compile()
    return nc


def pack_weights(Wd):
    """Wd [128, 64, 3, 3] f32 -> lhsT blocks [128, 18*32] bf16.
    Per gate pair: 3 full blocks gate a (center kx=1, shift kx=0), 3 full
    blocks gate b (center kx=2, shift kx=1), 3 split blocks (rows 0-63 =
    gate a kx=2 via center; rows 64-127 = gate b kx=0 via shift)."""
    wp = np.zeros((128, 18, 32), np.float32)
    for pg in range(2):
        Wa = Wd[(2 * pg) * 32 : (2 * pg + 1) * 32]      # [32(m), 64, 3, 3]
        Wb_ = Wd[(2 * pg + 1) * 32 : (2 * pg + 2) * 32]
        for ky in range(3):
            blk = wp[:, pg * 9 + ky, :]
            blk[0:32, :] = Wa[:, 0:32, ky, 1].T
            blk[32:64, :] = Wa[:, 32:64, ky, 1].T
            blk[64:96, :] = Wa[:, 0:32, ky, 0].T
            blk[96:128, :] = Wa[:, 32:64, ky, 0].T
            blk = wp[:, pg * 9 + 3 + ky, :]
            blk[0:32, :] = Wb_[:, 0:32, ky, 2].T
            blk[32:64, :] = Wb_[:, 32:64, ky, 2].T
            blk[64:96, :] = Wb_[:, 0:32, ky, 1].T
            blk[96:128, :] = Wb_[:, 32:64, ky, 1].T
            blk = wp[:, pg * 9 + 6 + ky, :]
            blk[0:32, :] = Wa[:, 0:32, ky, 2].T
            blk[32:64, :] = Wa[:, 32:64, ky, 2].T
            blk[64:96, :] = Wb_[:, 0:32, ky, 0].T
            blk[96:128, :] = Wb_[:, 32:64, ky, 0].T
    return wp.reshape(128, 18 * 32).astype(ml_dtypes.bfloat16)


def pack_bias(bd):
    """bd [128] f32 -> [128, 4] f32 (partition p = 32*tile + ch)."""
    bp = np.zeros((128, 4), np.float32)
    for g in range(4):
        bp[:, g] = np.tile(bd[g * 32 : (g + 1) * 32], 4)
    return bp


def make_in_maps(x, Wf, bf, Wb, bb, n_steps=S):
    wpacks = [pack_weights(np.asarray(Wf, np.float32)),
              pack_weights(np.asarray(Wb, np.float32))]
    bpacks = [pack_bias(np.asarray(bf, np.float32)),
              pack_bias(np.asarray(bb, np.float32))]
    x = np.asarray(x, np.float32)
    in_maps = []
    for k in range(N_CORES):
        d, b = k // 4, k % 4
        xc = x[:n_steps, b] if d == 0 else x[::-1][:n_steps, b]
        xp = np.zeros((n_steps, CIN, HP, WP), ml_dtypes.bfloat16)
        xp[:, :, 1 : H + 1, 1 : W + 1] = xc
        p0 = np.zeros((128, HP, WP), ml_dtypes.bfloat16)
        p0[0:32] = xp[0]
        p0[64:96, :, 1:] = xp[0, :, :, :-1]
        in_maps.append(
            {
                "x": xp.reshape(n_steps, CIN, PADN),
                "w": wpacks[d],
                "bias": bpacks[d],
                "plane0": p0.reshape(128, PADN),
            }
        )
    return in_maps


_CACHED_NC = {}


def kernel(x, Wf, bf, Wb, bb):
    zero_bias = (not np.any(np.asarray(bf))) and (not np.any(np.asarray(bb)))
    nc = _CACHED_NC.get(zero_bias)
    if nc is None:
        nc = _CACHED_NC[zero_bias] = build_program(S, zero_bias)
    in_maps = make_in_maps(x, Wf, bf, Wb, bb)
    res = run_bass_kernel_spmd(nc, in_maps, core_ids=list(range(N_CORES)))
    out = np.empty((S, B, 2 * CO, H, W), np.float32)
    for k in range(N_CORES):
        d, b = k // 4, k % 4
        yk = np.asarray(res.results[k]["y"], dtype=np.float32)
        # [S, (j c), (q g r w)] -> [S, c, q, g, r, j, w]; row = 64q+16g+4r+j
        yk = yk.reshape(S, TPG, CO, 2, 4, 4, W).transpose(0, 2, 3, 4, 5, 1, 6)
        out[:, b, d * CO : (d + 1) * CO] = yk.reshape(S, CO, H, W)
    return out


if __name__ == "__main__":
    import jax

    jax.config.update("jax_platforms", "cpu")
    rng = np.random.default_rng(0)
    x = rng.standard_normal((S, B, CIN, H, W), np.float32)
    Wf = (rng.standard_normal((128, 64, 3, 3)) * 0.05).astype(np.float32)
    Wb = (rng.standard_normal((128, 64, 3, 3)) * 0.05).astype(np.float32)
    bf = np.zeros(128, np.float32)
    bb = np.zeros(128, np.float32)
    y = kernel(x, Wf, bf, Wb, bb)
    print("out", y.shape, y.dtype)


# revision 35
# speedup vs baseline: 1.1914x; 1.0071x over previous
"""Bidirectional ConvLSTM block for Trainium2 (Bass/Tile), 8-core SPMD.

Problem: x [S=16, B=4, Cin=32, H=128, W=128] f32, Wf/Wb [128, 64, 3, 3],
bf/bb [128].  Output [S, B, 2*Co=64, H, W]: forward ConvLSTM hidden states
concat backward ConvLSTM (run on time-reversed x, not re-flipped).

Sharding: 8 independent recurrences = 2 directions x 4 batch elements.
Core k runs direction d=k//4 on batch b=k%4.  No cross-core communication.

Per-core kernel design (v2):
  - SBUF "act" tile [128 part, 130*130] bf16 (ping/pong): zero-padded
    (H+2)x(W+2) spatial plane per channel.
      partitions  0-31 : x_t   (center copy)
      partitions 32-63 : h_{t-1} (center copy)
      partitions 64-95 : x_t   shifted right by one column
      partitions 96-127: h_{t-1} shifted
    3x3 conv => 6 matmul passes per gate: 3 passes pair (dy,0)+(dy,-1)
    via the shifted rows; 3 passes do (dy,+1) with zero weights on them.
  - Per group (16 image rows = 4 spatial tiles x 512 positions): one PSUM
    tile [128, 2048] (4 banks), gate g in cols 512g.  Col-tiled matmuls
    (tile_position (0,32j)) write [32j:32j+32, 512g:...]: partitions =
    32*tile + channel, so pointwise runs on full 128-partition tiles.
    psum pool bufs=2 -> groups double-buffer 4+4 banks, PE never waits.
  - All dma_starts ride the Sync queue (plus x loads); the Scalar queue
    carries ONLY activations (v1 put h-scatter DMAs there, each DIRECT2D
    blocking the act FIFO ~0.7-6.5us -> PE starve -> HAM re-throttle).
  - y stored bf16 in native SBUF order [S, group, 128, 512] with one
    contiguous DMA per group; host reassembles/upcasts.
  - h written once (bf16): y DMA + 2 merged plane scatters per group.
"""

import os
import sys

import numpy as np

for _p in ("/opt/trn_rl_repo", "/root/.axon_site/_ro/trn_rl_repo"):
    if os.path.isdir(_p) and _p not in sys.path:
        sys.path.insert(0, _p)

import ml_dtypes  # noqa: E402
import concourse.bass as bass  # noqa: E402,F401
import concourse.mybir as mybir  # noqa: E402
from concourse import bacc, tile  # noqa: E402
from concourse.bass_utils import run_bass_kernel_spmd  # noqa: E402

F32 = mybir.dt.float32
BF16 = mybir.dt.bfloat16
AF = mybir.ActivationFunctionType

S, B, CIN, H, W = 16, 4, 32, 128, 128
CO = 32
HP, WP = H + 2, W + 2          # 130 x 130 padded plane
PADN = HP * WP                 # 16900
NSP = H * W                    # 16384
NT = 512                       # spatial positions per matmul tile (4 rows)
TPG = 4                        # tiles per group (col-tiled together)
GROUPS = NSP // (NT * TPG)     # 8 groups per step; group = 16 image rows
N_CORES = 8


def build_kernel(nc, tc, x_ap, w_ap, b_ap, y_ap, p0_ap, n_steps, zero_bias):
    ctx_pools = []

    def pool(**kw):
        p = tc.tile_pool(**kw)
        ctx_pools.append(p)
        return p.__enter__()

    const = pool(name="const", bufs=1)
    tmp = pool(name="tmp", bufs=3)
    psum = pool(name="psum", bufs=2, space="PSUM")

    # Persistent tiles
    a0 = const.tile([128, PADN], BF16, tag="act0")
    a1 = const.tile([128, PADN], BF16, tag="act1")
    acts = [a0, a1]
    ctile = const.tile([128, GROUPS * NT], F32, tag="c")
    wsb = const.tile([128, 18 * 32], BF16, tag="w")
    bsb = const.tile([128, 4], F32, tag="bias")

    # Step-0 act plane comes fully host-built (x copies + zeroed h + pads),
    # loaded in row chunks so group 0's matmuls start after ~19 rows land.
    nc.sync.dma_start(wsb[:, :], w_ap)
    nc.sync.dma_start(a0[:, 0 : 19 * WP], p0_ap[:, 0 : 19 * WP])
    for lo, hi in ((19, 51), (51, 83), (83, HP)):
        nc.sync.dma_start(a0[:, lo * WP : hi * WP], p0_ap[:, lo * WP : hi * WP])
    nc.sync.dma_start(bsb[:, :], b_ap)

    # --- one-time zero init (a1 h-region borders only; its x regions are
    # re-loaded host-padded every step, h interiors are scatter-written) ---
    ar1 = a1.rearrange("p (r w) -> p r w", r=HP)
    nc.gpsimd.memset(a1[32:64, 0:WP], 0.0)                  # pad row 0
    nc.gpsimd.memset(a1[96:128, 0:WP], 0.0)
    nc.gpsimd.memset(a1[32:64, (HP - 1) * WP :], 0.0)       # pad row 129
    nc.gpsimd.memset(a1[96:128, (HP - 1) * WP :], 0.0)
    nc.gpsimd.memset(ar1[32:64, :, 0:1], 0.0)               # col 0 (unread, keep finite)
    nc.gpsimd.memset(ar1[32:64, :, WP - 1 : WP], 0.0)       # center col 129
    nc.gpsimd.memset(ar1[96:128, :, 0:2], 0.0)              # shift cols 0-1
    nc.vector.memset(ctile[:, :], 0.0)

    def load_x(t):
        # x arrives host-padded to the full 130x130 plane: both the center
        # copy and the +1-shifted copy are single contiguous runs per
        # partition (the shift picks up its zero border from the host pad).
        nc.sync.dma_start(acts[t % 2][0:32, :], x_ap[t])
        nc.sync.dma_start(acts[t % 2][64:96, 1:PADN], x_ap[t, :, 0 : PADN - 1])

    from concourse.ap import AP as _AP

    def scatter_ap(a_nxt, pbase, q, j, coloff):
        """3D dst AP: partitions pbase..pbase+32, rows 64q+j+1 + 4k (k=0..16),
        cols coloff..coloff+128 of the 130x130 plane.  (Spatial tile j holds
        rows == j mod 4 of its quad, so the 16 staged rows form one stride-4
        run.)"""
        base = a_nxt[:, :]
        base_row = 64 * q + j + 1
        off = pbase * PADN + base_row * WP + coloff
        return _AP(
            base.tensor,
            base.offset + off,
            [[PADN, 32], [4 * WP, 16], [1, W]],
        )

    for t in range(n_steps):
        a_cur = acts[t % 2]
        a_nxt = acts[(t + 1) % 2]
        ar_cur = a_cur.rearrange("p (r w) -> p r w", r=HP)
        if t + 1 < n_steps:
            load_x(t + 1)

        hq = None
        pending = None
        for grp in range(GROUPS):
            if grp % 4 == 0:
                hq = tmp.tile(
                    [128, 4 * NT], BF16, tag="hq", name=f"hq{t}_{grp // 4}"
                )
            # Hybrid pass schedule, 18 slot-times per group (vs 24 all-full):
            # gate 2p: 3 full K=128 passes pairing (dy,0)+(dy,-1);
            # gate 2p+1: 3 full passes at col offset +1 pairing (dy,+1)+(dy,0);
            # leftovers -- 2p's (dy,+1) and 2p+1's (dy,-1) -- share 3 row-
            # split K=64 slots (center half / shifted half run concurrently
            # into their own PSUM banks via row tiling).
            zt = psum.tile([128, 4 * NT], F32, tag="z", name=f"z{t}_{grp}")
            for pg in range(2):
                ga, gb = 2 * pg, 2 * pg + 1
                for half, g in ((0, ga), (1, gb)):
                    for p3 in range(3):
                        dy = p3 - 1
                        blk = (pg * 9 + half * 3 + p3) * 32
                        for j in range(TPG):
                            r0 = 16 * grp + j + 1 + dy
                            nc.tensor.matmul(
                                zt[32 * j : 32 * j + 32, g * NT : (g + 1) * NT],
                                wsb[:, blk : blk + 32],
                                ar_cur[:, r0 : r0 + 13 : 4, 1 + half : W + 1 + half],
                                start=(p3 == 0),
                                stop=False,
                                skip_group_check=True,
                                tile_position=(0, 32 * j),
                            )
                for p3 in range(3):
                    dy = p3 - 1
                    blk = (pg * 9 + 6 + p3) * 32
                    for j in range(TPG):
                        r0 = 16 * grp + j + 1 + dy
                        rows = slice(r0, r0 + 13, 4)
                        nc.tensor.matmul(
                            zt[32 * j : 32 * j + 32, ga * NT : (ga + 1) * NT],
                            wsb[0:64, blk : blk + 32],
                            ar_cur[0:64, rows, 2 : W + 2],
                            start=False,
                            stop=(p3 == 2),
                            skip_group_check=True,
                            tile_position=(0, 32 * j),
                        )
                        nc.tensor.matmul(
                            zt[32 * j : 32 * j + 32, gb * NT : (gb + 1) * NT],
                            wsb[64:128, blk : blk + 32],
                            ar_cur[64:128, rows, 1 : W + 1],
                            start=False,
                            stop=(p3 == 2),
                            skip_group_check=True,
                            tile_position=(64, 32 * j),
                        )

            # ---- pointwise ----
            csl = ctile[:, grp * NT : (grp + 1) * NT]
            sio = tmp.tile([128, 3 * NT], BF16, tag="sio", name=f"sio{t}_{grp}")
            tg = tmp.tile([128, NT], BF16, tag="tg", name=f"tg{t}_{grp}")
            if zero_bias:
                nc.scalar.activation(sio[:, :], zt[:, 0 : 3 * NT], AF.Sigmoid)
                nc.scalar.activation(tg[:, :], zt[:, 3 * NT :], AF.Tanh)
            else:
                for g, sl in ((0, 0), (1, 1), (2, 2)):
                    nc.scalar.activation(
                        sio[:, sl * NT : (sl + 1) * NT],
                        zt[:, g * NT : (g + 1) * NT],
                        AF.Sigmoid,
                        bias=bsb[:, g : g + 1],
                    )
                nc.scalar.activation(
                    tg[:, :], zt[:, 3 * NT :], AF.Tanh, bias=bsb[:, 3:4]
                )
            si = sio[:, 0:NT]
            sf = sio[:, NT : 2 * NT]
            so = sio[:, 2 * NT : 3 * NT]

            t2 = tmp.tile([128, NT], F32, tag="t2", name=f"t2_{t}_{grp}")
            t3 = tmp.tile([128, NT], F32, tag="t3", name=f"t3_{t}_{grp}")
            nc.vector.tensor_mul(t3[:, :], sf, csl)
            nc.vector.tensor_mul(t2[:, :], si, tg[:, :])
            nc.vector.tensor_add(csl, t2[:, :], t3[:, :])

            # phase 2 (deferred one group): tanh(c), h, y, scatters.  Issuing
            # the previous group's tanh_c AFTER this group's gate acts keeps
            # the Scalar FIFO (and the next group's matmuls, which track the
            # sigmoid) from stalling on the DVE c-update chain.
            def flush(pgrp, phq, pcsl, pso):
                tcn = tmp.tile([128, NT], BF16, tag="tcn", name=f"tcn{t}_{pgrp}")
                nc.scalar.activation(tcn[:, :], pcsl, AF.Tanh)
                hsl = phq[:, (pgrp % 4) * NT : (pgrp % 4 + 1) * NT]
                nc.vector.tensor_mul(hsl, pso, tcn[:, :])
                if t == n_steps - 1:
                    nc.sync.dma_start(
                        y_ap[t, :, pgrp * NT : (pgrp + 1) * NT], hsl
                    )
                if pgrp % 4 == 3:
                    q = pgrp // 4
                    if t < n_steps - 1:
                        nc.sync.dma_start(
                            y_ap[t, :, q * 4 * NT : (q + 1) * 4 * NT], phq[:, :]
                        )
                    if t + 1 < n_steps:
                        for j in range(TPG):
                            src = phq[32 * j : 32 * j + 32, :]
                            nc.sync.dma_start(
                                scatter_ap(a_nxt, 32, q, j, 1), src
                            )
                            nc.sync.dma_start(
                                scatter_ap(a_nxt, 96, q, j, 2), src
                            )

            if pending is not None:
                flush(*pending)
            pending = (grp, hq, csl, so)
        flush(*pending)

    for p in reversed(ctx_pools):
        p.__exit__(None, None, None)


def build_program(n_steps=S, zero_bias=True):
    nc = bacc.Bacc(
        "TRN2",
        target_bir_lowering=False,
        debug=False,
        enable_asserts=False,
        num_devices=N_CORES,
    )
    x_d = nc.dram_tensor("x", [n_steps, CIN, PADN], BF16, kind="ExternalInput")
    w_d = nc.dram_tensor("w", [128, 18 * 32], BF16, kind="ExternalInput")
    b_d = nc.dram_tensor("bias", [128, 4], F32, kind="ExternalInput")
    y_d = nc.dram_tensor("y", [n_steps, 128, GROUPS * NT], BF16, kind="ExternalOutput")
    p0_d = nc.dram_tensor("plane0", [128, PADN], BF16, kind="ExternalInput")
    with tile.TileContext(nc) as tc:
        build_kernel(
            nc, tc, x_d.ap(), w_d.ap(), b_d.ap(), y_d.ap(), p0_d.ap(),
            n_steps, zero_bias,
        )
    nc.compile()
    return nc


def pack_weights(Wd):
    """Wd [128, 64, 3, 3] f32 -> lhsT blocks [128, 18*32] bf16.
    Per gate pair: 3 full blocks gate a (center kx=1, shift kx=0), 3 full
    blocks gate b (center kx=2, shift kx=1), 3 split blocks (rows 0-63 =
    gate a kx=2 via center; rows 64-127 = gate b kx=0 via shift)."""
    wp = np.zeros((128, 18, 32), np.float32)
    for pg in range(2):
        Wa = Wd[(2 * pg) * 32 : (2 * pg + 1) * 32]      # [32(m), 64, 3, 3]
        Wb_ = Wd[(2 * pg + 1) * 32 : (2 * pg + 2) * 32]
        for ky in range(3):
            blk = wp[:, pg * 9 + ky, :]
            blk[0:32, :] = Wa[:, 0:32, ky, 1].T
            blk[32:64, :] = Wa[:, 32:64, ky, 1].T
            blk[64:96, :] = Wa[:, 0:32, ky, 0].T
            blk[96:128, :] = Wa[:, 32:64, ky, 0].T
            blk = wp[:, pg * 9 + 3 + ky, :]
            blk[0:32, :] = Wb_[:, 0:32, ky, 2].T
            blk[32:64, :] = Wb_[:, 32:64, ky, 2].T
            blk[64:96, :] = Wb_[:, 0:32, ky, 1].T
            blk[96:128, :] = Wb_[:, 32:64, ky, 1].T
            blk = wp[:, pg * 9 + 6 + ky, :]
            blk[0:32, :] = Wa[:, 0:32, ky, 2].T
            blk[32:64, :] = Wa[:, 32:64, ky, 2].T
            blk[64:96, :] = Wb_[:, 0:32, ky, 0].T
            blk[96:128, :] = Wb_[:, 32:64, ky, 0].T
    return wp.reshape(128, 18 * 32).astype(ml_dtypes.bfloat16)


def pack_bias(bd):
    """bd [128] f32 -> [128, 4] f32 (partition p = 32*tile + ch)."""
    bp = np.zeros((128, 4), np.float32)
    for g in range(4):
        bp[:, g] = np.tile(bd[g * 32 : (g + 1) * 32], 4)
    return bp


def make_in_maps(x, Wf, bf, Wb, bb, n_steps=S):
    wpacks = [pack_weights(np.asarray(Wf, np.float32)),
              pack_weights(np.asarray(Wb, np.float32))]
    bpacks = [pack_bias(np.asarray(bf, np.float32)),
              pack_bias(np.asarray(bb, np.float32))]
    x = np.asarray(x, np.float32)
    in_maps = []
    for k in range(N_CORES):
        d, b = k // 4, k % 4
        xc = x[:n_steps, b] if d == 0 else x[::-1][:n_steps, b]
        xp = np.zeros((n_steps, CIN, HP, WP), ml_dtypes.bfloat16)
        xp[:, :, 1 : H + 1, 1 : W + 1] = xc
        p0 = np.zeros((128, HP, WP), ml_dtypes.bfloat16)
        p0[0:32] = xp[0]
        p0[64:96, :, 1:] = xp[0, :, :, :-1]
        in_maps.append(
            {
                "x": xp.reshape(n_steps, CIN, PADN),
                "w": wpacks[d],
                "bias": bpacks[d],
                "plane0": p0.reshape(128, PADN),
            }
        )
    return in_maps


_CACHED_NC = {}


def kernel(x, Wf, bf, Wb, bb):
    zero_bias = (not np.any(np.asarray(bf))) and (not np.any(np.asarray(bb)))
    nc = _CACHED_NC.get(zero_bias)
    if nc is None:
        nc = _CACHED_NC[zero_bias] = build_program(S, zero_bias)
    in_maps = make_in_maps(x, Wf, bf, Wb, bb)
    res = run_bass_kernel_spmd(nc, in_maps, core_ids=list(range(N_CORES)))
    out = np.empty((S, B, 2 * CO, H, W), np.float32)
    for k in range(N_CORES):
        d, b = k // 4, k % 4
        yk = np.asarray(res.results[k]["y"], dtype=np.float32)
        # [S, (j c), (q g r w)] -> [S, c, q, g, r, j, w]; row = 64q+16g+4r+j
        yk = yk.reshape(S, TPG, CO, 2, 4, 4, W).transpose(0, 2, 3, 4, 5, 1, 6)
        out[:, b, d * CO : (d + 1) * CO] = yk.reshape(S, CO, H, W)
    return out


if __name__ == "__main__":
    import jax

    jax.config.update("jax_platforms", "cpu")
    rng = np.random.default_rng(0)
    x = rng.standard_normal((S, B, CIN, H, W), np.float32)
    Wf = (rng.standard_normal((128, 64, 3, 3)) * 0.05).astype(np.float32)
    Wb = (rng.standard_normal((128, 64, 3, 3)) * 0.05).astype(np.float32)
    bf = np.zeros(128, np.float32)
    bb = np.zeros(128, np.float32)
    y = kernel(x, Wf, bf, Wb, bb)
    print("out", y.shape, y.dtype)


# revision 37
# speedup vs baseline: 1.2010x; 1.0081x over previous
"""Bidirectional ConvLSTM block for Trainium2 (Bass/Tile), 8-core SPMD.

Problem: x [S=16, B=4, Cin=32, H=128, W=128] f32, Wf/Wb [128, 64, 3, 3],
bf/bb [128].  Output [S, B, 2*Co=64, H, W]: forward ConvLSTM hidden states
concat backward ConvLSTM (run on time-reversed x, not re-flipped).

Sharding: 8 independent recurrences = 2 directions x 4 batch elements.
Core k runs direction d=k//4 on batch b=k%4.  No cross-core communication.

Per-core kernel design (v2):
  - SBUF "act" tile [128 part, 130*130] bf16 (ping/pong): zero-padded
    (H+2)x(W+2) spatial plane per channel.
      partitions  0-31 : x_t   (center copy)
      partitions 32-63 : h_{t-1} (center copy)
      partitions 64-95 : x_t   shifted right by one column
      partitions 96-127: h_{t-1} shifted
    3x3 conv => 6 matmul passes per gate: 3 passes pair (dy,0)+(dy,-1)
    via the shifted rows; 3 passes do (dy,+1) with zero weights on them.
  - Per group (16 image rows = 4 spatial tiles x 512 positions): one PSUM
    tile [128, 2048] (4 banks), gate g in cols 512g.  Col-tiled matmuls
    (tile_position (0,32j)) write [32j:32j+32, 512g:...]: partitions =
    32*tile + channel, so pointwise runs on full 128-partition tiles.
    psum pool bufs=2 -> groups double-buffer 4+4 banks, PE never waits.
  - All dma_starts ride the Sync queue (plus x loads); the Scalar queue
    carries ONLY activations (v1 put h-scatter DMAs there, each DIRECT2D
    blocking the act FIFO ~0.7-6.5us -> PE starve -> HAM re-throttle).
  - y stored bf16 in native SBUF order [S, group, 128, 512] with one
    contiguous DMA per group; host reassembles/upcasts.
  - h written once (bf16): y DMA + 2 merged plane scatters per group.
"""

import os
import sys

import numpy as np

for _p in ("/opt/trn_rl_repo", "/root/.axon_site/_ro/trn_rl_repo"):
    if os.path.isdir(_p) and _p not in sys.path:
        sys.path.insert(0, _p)

import ml_dtypes  # noqa: E402
import concourse.bass as bass  # noqa: E402,F401
import concourse.mybir as mybir  # noqa: E402
from concourse import bacc, tile  # noqa: E402
from concourse.bass_utils import run_bass_kernel_spmd  # noqa: E402

F32 = mybir.dt.float32
BF16 = mybir.dt.bfloat16
AF = mybir.ActivationFunctionType

S, B, CIN, H, W = 16, 4, 32, 128, 128
CO = 32
HP, WP = H + 2, W + 2          # 130 x 130 padded plane
PADN = HP * WP                 # 16900
NSP = H * W                    # 16384
NT = 512                       # spatial positions per matmul tile (4 rows)
TPG = 4                        # tiles per group (col-tiled together)
GROUPS = NSP // (NT * TPG)     # 8 groups per step; group = 16 image rows
N_CORES = 8


def build_kernel(nc, tc, x_ap, w_ap, b_ap, y_ap, p0_ap, n_steps, zero_bias):
    ctx_pools = []

    def pool(**kw):
        p = tc.tile_pool(**kw)
        ctx_pools.append(p)
        return p.__enter__()

    const = pool(name="const", bufs=1)
    tmp = pool(name="tmp", bufs=3)
    psum = pool(name="psum", bufs=2, space="PSUM")

    # Persistent tiles
    a0 = const.tile([128, PADN], BF16, tag="act0")
    a1 = const.tile([128, PADN], BF16, tag="act1")
    acts = [a0, a1]
    ctile = const.tile([128, GROUPS * NT], F32, tag="c")
    wsb = const.tile([128, 18 * 32], BF16, tag="w")
    bsb = const.tile([128, 4], F32, tag="bias")

    # Step-0 act plane comes fully host-built (x copies + zeroed h + pads),
    # loaded in row chunks so group 0's matmuls start after ~19 rows land.
    nc.sync.dma_start(wsb[:, :], w_ap)
    nc.sync.dma_start(a0[:, 0 : 19 * WP], p0_ap[:, 0 : 19 * WP])
    for lo, hi in ((19, 51), (51, 83), (83, HP)):
        nc.sync.dma_start(a0[:, lo * WP : hi * WP], p0_ap[:, lo * WP : hi * WP])
    nc.sync.dma_start(bsb[:, :], b_ap)

    # --- one-time zero init (a1 h-region borders only; its x regions are
    # re-loaded host-padded every step, h interiors are scatter-written) ---
    ar1 = a1.rearrange("p (r w) -> p r w", r=HP)
    nc.gpsimd.memset(a1[32:64, 0:WP], 0.0)                  # pad row 0
    nc.gpsimd.memset(a1[96:128, 0:WP], 0.0)
    nc.gpsimd.memset(a1[32:64, (HP - 1) * WP :], 0.0)       # pad row 129
    nc.gpsimd.memset(a1[96:128, (HP - 1) * WP :], 0.0)
    nc.gpsimd.memset(ar1[32:64, :, 0:1], 0.0)               # col 0 (unread, keep finite)
    nc.gpsimd.memset(ar1[32:64, :, WP - 1 : WP], 0.0)       # center col 129
    nc.gpsimd.memset(ar1[96:128, :, 0:2], 0.0)              # shift cols 0-1
    nc.vector.memset(ctile[:, :], 0.0)

    def load_x(t):
        # x arrives host-padded to the full 130x130 plane: both the center
        # copy and the +1-shifted copy are single contiguous runs per
        # partition (the shift picks up its zero border from the host pad).
        nc.sync.dma_start(acts[t % 2][0:32, :], x_ap[t])
        nc.sync.dma_start(acts[t % 2][64:96, 1:PADN], x_ap[t, :, 0 : PADN - 1])

    from concourse.ap import AP as _AP

    def scatter_ap(a_nxt, pbase, q, j, coloff):
        """3D dst AP: partitions pbase..pbase+32, rows 64q+j+1 + 4k (k=0..16),
        cols coloff..coloff+128 of the 130x130 plane.  (Spatial tile j holds
        rows == j mod 4 of its quad, so the 16 staged rows form one stride-4
        run.)"""
        base = a_nxt[:, :]
        base_row = 64 * q + j + 1
        off = pbase * PADN + base_row * WP + coloff
        return _AP(
            base.tensor,
            base.offset + off,
            [[PADN, 32], [4 * WP, 16], [1, W]],
        )

    def pair_scatter_ap(a_nxt, pbase, pr, j, coloff):
        """Like scatter_ap but for one pair of groups: rows 32pr+j+1 + 4k,
        k=0..8."""
        base = a_nxt[:, :]
        off = pbase * PADN + (32 * pr + j + 1) * WP + coloff
        return _AP(
            base.tensor,
            base.offset + off,
            [[PADN, 32], [4 * WP, 8], [1, W]],
        )

    for t in range(n_steps):
        a_cur = acts[t % 2]
        a_nxt = acts[(t + 1) % 2]
        ar_cur = a_cur.rearrange("p (r w) -> p r w", r=HP)
        if t + 1 < n_steps:
            load_x(t + 1)

        hq = None
        pending = None
        for grp in range(GROUPS):
            if grp % 4 == 0:
                hq = tmp.tile(
                    [128, 4 * NT], BF16, tag="hq", name=f"hq{t}_{grp // 4}"
                )
            # Hybrid pass schedule, 18 slot-times per group (vs 24 all-full):
            # gate 2p: 3 full K=128 passes pairing (dy,0)+(dy,-1);
            # gate 2p+1: 3 full passes at col offset +1 pairing (dy,+1)+(dy,0);
            # leftovers -- 2p's (dy,+1) and 2p+1's (dy,-1) -- share 3 row-
            # split K=64 slots (center half / shifted half run concurrently
            # into their own PSUM banks via row tiling).
            zt = psum.tile([128, 4 * NT], F32, tag="z", name=f"z{t}_{grp}")
            for pg in range(2):
                ga, gb = 2 * pg, 2 * pg + 1
                for half, g in ((0, ga), (1, gb)):
                    for p3 in range(3):
                        dy = p3 - 1
                        blk = (pg * 9 + half * 3 + p3) * 32
                        for j in range(TPG):
                            r0 = 16 * grp + j + 1 + dy
                            nc.tensor.matmul(
                                zt[32 * j : 32 * j + 32, g * NT : (g + 1) * NT],
                                wsb[:, blk : blk + 32],
                                ar_cur[:, r0 : r0 + 13 : 4, 1 + half : W + 1 + half],
                                start=(p3 == 0),
                                stop=False,
                                skip_group_check=True,
                                tile_position=(0, 32 * j),
                            )
                for p3 in range(3):
                    dy = p3 - 1
                    blk = (pg * 9 + 6 + p3) * 32
                    for j in range(TPG):
                        r0 = 16 * grp + j + 1 + dy
                        rows = slice(r0, r0 + 13, 4)
                        nc.tensor.matmul(
                            zt[32 * j : 32 * j + 32, ga * NT : (ga + 1) * NT],
                            wsb[0:64, blk : blk + 32],
                            ar_cur[0:64, rows, 2 : W + 2],
                            start=False,
                            stop=(p3 == 2),
                            skip_group_check=True,
                            tile_position=(0, 32 * j),
                        )
                        nc.tensor.matmul(
                            zt[32 * j : 32 * j + 32, gb * NT : (gb + 1) * NT],
                            wsb[64:128, blk : blk + 32],
                            ar_cur[64:128, rows, 1 : W + 1],
                            start=False,
                            stop=(p3 == 2),
                            skip_group_check=True,
                            tile_position=(64, 32 * j),
                        )

            # ---- pointwise ----
            csl = ctile[:, grp * NT : (grp + 1) * NT]
            sio = tmp.tile([128, 3 * NT], BF16, tag="sio", name=f"sio{t}_{grp}")
            tg = tmp.tile([128, NT], BF16, tag="tg", name=f"tg{t}_{grp}")
            if zero_bias:
                nc.scalar.activation(sio[:, :], zt[:, 0 : 3 * NT], AF.Sigmoid)
                nc.scalar.activation(tg[:, :], zt[:, 3 * NT :], AF.Tanh)
            else:
                for g, sl in ((0, 0), (1, 1), (2, 2)):
                    nc.scalar.activation(
                        sio[:, sl * NT : (sl + 1) * NT],
                        zt[:, g * NT : (g + 1) * NT],
                        AF.Sigmoid,
                        bias=bsb[:, g : g + 1],
                    )
                nc.scalar.activation(
                    tg[:, :], zt[:, 3 * NT :], AF.Tanh, bias=bsb[:, 3:4]
                )
            si = sio[:, 0:NT]
            sf = sio[:, NT : 2 * NT]
            so = sio[:, 2 * NT : 3 * NT]

            t2 = tmp.tile([128, NT], F32, tag="t2", name=f"t2_{t}_{grp}")
            t3 = tmp.tile([128, NT], F32, tag="t3", name=f"t3_{t}_{grp}")
            nc.vector.tensor_mul(t3[:, :], sf, csl)
            nc.vector.tensor_mul(t2[:, :], si, tg[:, :])
            nc.vector.tensor_add(csl, t2[:, :], t3[:, :])

            # phase 2 (deferred one group): tanh(c), h, y, scatters.  Issuing
            # the previous group's tanh_c AFTER this group's gate acts keeps
            # the Scalar FIFO (and the next group's matmuls, which track the
            # sigmoid) from stalling on the DVE c-update chain.
            def flush(pgrp, phq, pcsl, pso):
                tcn = tmp.tile([128, NT], BF16, tag="tcn", name=f"tcn{t}_{pgrp}")
                nc.scalar.activation(tcn[:, :], pcsl, AF.Tanh)
                hsl = phq[:, (pgrp % 4) * NT : (pgrp % 4 + 1) * NT]
                nc.vector.tensor_mul(hsl, pso, tcn[:, :])
                if t == n_steps - 1:
                    nc.sync.dma_start(
                        y_ap[t, :, pgrp * NT : (pgrp + 1) * NT], hsl
                    )
                if pgrp % 4 == 3 and t < n_steps - 1:
                    q = pgrp // 4
                    nc.sync.dma_start(
                        y_ap[t, :, q * 4 * NT : (q + 1) * 4 * NT], phq[:, :]
                    )
                if pgrp % 2 == 1 and t + 1 < n_steps:
                    # scatter per PAIR of groups (half the burst of per-quad):
                    # the 9-DMA quad burst contended SBUF with the rhs stream
                    pr = pgrp // 2
                    bc = ((pgrp % 4) - 1) * NT
                    for j in range(TPG):
                        src = phq[32 * j : 32 * j + 32, bc : bc + 2 * NT]
                        nc.sync.dma_start(pair_scatter_ap(a_nxt, 32, pr, j, 1), src)
                        nc.sync.dma_start(pair_scatter_ap(a_nxt, 96, pr, j, 2), src)

            if pending is not None:
                flush(*pending)
            pending = (grp, hq, csl, so)
        flush(*pending)

    for p in reversed(ctx_pools):
        p.__exit__(None, None, None)


def build_program(n_steps=S, zero_bias=True):
    nc = bacc.Bacc(
        "TRN2",
        target_bir_lowering=False,
        debug=False,
        enable_asserts=False,
        num_devices=N_CORES,
    )
    x_d = nc.dram_tensor("x", [n_steps, CIN, PADN], BF16, kind="ExternalInput")
    w_d = nc.dram_tensor("w", [128, 18 * 32], BF16, kind="ExternalInput")
    b_d = nc.dram_tensor("bias", [128, 4], F32, kind="ExternalInput")
    y_d = nc.dram_tensor("y", [n_steps, 128, GROUPS * NT], BF16, kind="ExternalOutput")
    p0_d = nc.dram_tensor("plane0", [128, PADN], BF16, kind="ExternalInput")
    with tile.TileContext(nc) as tc:
        build_kernel(
            nc, tc, x_d.ap(), w_d.ap(), b_d.ap(), y_d.ap(), p0_d.ap(),
            n_steps, zero_bias,
        )
    nc.compile()
    return nc


def pack_weights(Wd):
    """Wd [128, 64, 3, 3] f32 -> lhsT blocks [128, 18*32] bf16.
    Per gate pair: 3 full blocks gate a (center kx=1, shift kx=0), 3 full
    blocks gate b (center kx=2, shift kx=1), 3 split blocks (rows 0-63 =
    gate a kx=2 via center; rows 64-127 = gate b kx=0 via shift)."""
    wp = np.zeros((128, 18, 32), np.float32)
    for pg in range(2):
        Wa = Wd[(2 * pg) * 32 : (2 * pg + 1) * 32]      # [32(m), 64, 3, 3]
        Wb_ = Wd[(2 * pg + 1) * 32 : (2 * pg + 2) * 32]
        for ky in range(3):
            blk = wp[:, pg * 9 + ky, :]
            blk[0:32, :] = Wa[:, 0:32, ky, 1].T
            blk[32:64, :] = Wa[:, 32:64, ky, 1].T
            blk[64:96, :] = Wa[:, 0:32, ky, 0].T
            blk[96:128, :] = Wa[:, 32:64, ky, 0].T
            blk = wp[:, pg * 9 + 3 + ky, :]
            blk[0:32, :] = Wb_[:, 0:32, ky, 2].T
            blk[32:64, :] = Wb_[:, 32:64, ky, 2].T
            blk[64:96, :] = Wb_[:, 0:32, ky, 1].T
            blk[96:128, :] = Wb_[:, 32:64, ky, 1].T
            blk = wp[:, pg * 9 + 6 + ky, :]
            blk[0:32, :] = Wa[:, 0:32, ky, 2].T
            blk[32:64, :] = Wa[:, 32:64, ky, 2].T
            blk[64:96, :] = Wb_[:, 0:32, ky, 0].T
            blk[96:128, :] = Wb_[:, 32:64, ky, 0].T
    return wp.reshape(128, 18 * 32).astype(ml_dtypes.bfloat16)


def pack_bias(bd):
    """bd [128] f32 -> [128, 4] f32 (partition p = 32*tile + ch)."""
    bp = np.zeros((128, 4), np.float32)
    for g in range(4):
        bp[:, g] = np.tile(bd[g * 32 : (g + 1) * 32], 4)
    return bp


def make_in_maps(x, Wf, bf, Wb, bb, n_steps=S):
    wpacks = [pack_weights(np.asarray(Wf, np.float32)),
              pack_weights(np.asarray(Wb, np.float32))]
    bpacks = [pack_bias(np.asarray(bf, np.float32)),
              pack_bias(np.asarray(bb, np.float32))]
    x = np.asarray(x, np.float32)
    in_maps = []
    for k in range(N_CORES):
        d, b = k // 4, k % 4
        xc = x[:n_steps, b] if d == 0 else x[::-1][:n_steps, b]
        xp = np.zeros((n_steps, CIN, HP, WP), ml_dtypes.bfloat16)
        xp[:, :, 1 : H + 1, 1 : W + 1] = xc
        p0 = np.zeros((128, HP, WP), ml_dtypes.bfloat16)
        p0[0:32] = xp[0]
        p0[64:96, :, 1:] = xp[0, :, :, :-1]
        in_maps.append(
            {
                "x": xp.reshape(n_steps, CIN, PADN),
                "w": wpacks[d],
                "bias": bpacks[d],
                "plane0": p0.reshape(128, PADN),
            }
        )
    return in_maps


_CACHED_NC = {}


def kernel(x, Wf, bf, Wb, bb):
    zero_bias = (not np.any(np.asarray(bf))) and (not np.any(np.asarray(bb)))
    nc = _CACHED_NC.get(zero_bias)
    if nc is None:
        nc = _CACHED_NC[zero_bias] = build_program(S, zero_bias)
    in_maps = make_in_maps(x, Wf, bf, Wb, bb)
    res = run_bass_kernel_spmd(nc, in_maps, core_ids=list(range(N_CORES)))
    out = np.empty((S, B, 2 * CO, H, W), np.float32)
    for k in range(N_CORES):
        d, b = k // 4, k % 4
        yk = np.asarray(res.results[k]["y"], dtype=np.float32)
        # [S, (j c), (q g r w)] -> [S, c, q, g, r, j, w]; row = 64q+16g+4r+j
        yk = yk.reshape(S, TPG, CO, 2, 4, 4, W).transpose(0, 2, 3, 4, 5, 1, 6)
        out[:, b, d * CO : (d + 1) * CO] = yk.reshape(S, CO, H, W)
    return out


if __name__ == "__main__":
    import jax

    jax.config.update("jax_platforms", "cpu")
    rng = np.random.default_rng(0)
    x = rng.standard_normal((S, B, CIN, H, W), np.float32)
    Wf = (rng.standard_normal((128, 64, 3, 3)) * 0.05).astype(np.float32)
    Wb = (rng.standard_normal((128, 64, 3, 3)) * 0.05).astype(np.float32)
    bf = np.zeros(128, np.float32)
    bb = np.zeros(128, np.float32)
    y = kernel(x, Wf, bf, Wb, bb)
    print("out", y.shape, y.dtype)


# revision 38
# speedup vs baseline: 1.2161x; 1.0125x over previous
"""Bidirectional ConvLSTM block for Trainium2 (Bass/Tile), 8-core SPMD.

Problem: x [S=16, B=4, Cin=32, H=128, W=128] f32, Wf/Wb [128, 64, 3, 3],
bf/bb [128].  Output [S, B, 2*Co=64, H, W]: forward ConvLSTM hidden states
concat backward ConvLSTM (run on time-reversed x, not re-flipped).

Sharding: 8 independent recurrences = 2 directions x 4 batch elements.
Core k runs direction d=k//4 on batch b=k%4.  No cross-core communication.

Per-core kernel design (v2):
  - SBUF "act" tile [128 part, 130*130] bf16 (ping/pong): zero-padded
    (H+2)x(W+2) spatial plane per channel.
      partitions  0-31 : x_t   (center copy)
      partitions 32-63 : h_{t-1} (center copy)
      partitions 64-95 : x_t   shifted right by one column
      partitions 96-127: h_{t-1} shifted
    3x3 conv => 6 matmul passes per gate: 3 passes pair (dy,0)+(dy,-1)
    via the shifted rows; 3 passes do (dy,+1) with zero weights on them.
  - Per group (16 image rows = 4 spatial tiles x 512 positions): one PSUM
    tile [128, 2048] (4 banks), gate g in cols 512g.  Col-tiled matmuls
    (tile_position (0,32j)) write [32j:32j+32, 512g:...]: partitions =
    32*tile + channel, so pointwise runs on full 128-partition tiles.
    psum pool bufs=2 -> groups double-buffer 4+4 banks, PE never waits.
  - All dma_starts ride the Sync queue (plus x loads); the Scalar queue
    carries ONLY activations (v1 put h-scatter DMAs there, each DIRECT2D
    blocking the act FIFO ~0.7-6.5us -> PE starve -> HAM re-throttle).
  - y stored bf16 in native SBUF order [S, group, 128, 512] with one
    contiguous DMA per group; host reassembles/upcasts.
  - h written once (bf16): y DMA + 2 merged plane scatters per group.
"""

import os
import sys

import numpy as np

for _p in ("/opt/trn_rl_repo", "/root/.axon_site/_ro/trn_rl_repo"):
    if os.path.isdir(_p) and _p not in sys.path:
        sys.path.insert(0, _p)

import ml_dtypes  # noqa: E402
import concourse.bass as bass  # noqa: E402,F401
import concourse.mybir as mybir  # noqa: E402
from concourse import bacc, tile  # noqa: E402
from concourse.bass_utils import run_bass_kernel_spmd  # noqa: E402

F32 = mybir.dt.float32
BF16 = mybir.dt.bfloat16
AF = mybir.ActivationFunctionType

S, B, CIN, H, W = 16, 4, 32, 128, 128
CO = 32
HP, WP = H + 2, W + 2          # 130 x 130 padded plane
PADN = HP * WP                 # 16900
NSP = H * W                    # 16384
NT = 512                       # spatial positions per matmul tile (4 rows)
TPG = 4                        # tiles per group (col-tiled together)
GROUPS = NSP // (NT * TPG)     # 8 groups per step; group = 16 image rows
N_CORES = 8


def build_kernel(nc, tc, x_ap, w_ap, b_ap, y_ap, p0_ap, n_steps, zero_bias):
    ctx_pools = []

    def pool(**kw):
        p = tc.tile_pool(**kw)
        ctx_pools.append(p)
        return p.__enter__()

    const = pool(name="const", bufs=1)
    tmp = pool(name="tmp", bufs=3)
    psum = pool(name="psum", bufs=2, space="PSUM")

    # Persistent tiles
    a0 = const.tile([128, PADN], BF16, tag="act0")
    a1 = const.tile([128, PADN], BF16, tag="act1")
    acts = [a0, a1]
    ctile = const.tile([128, GROUPS * NT], F32, tag="c")
    wsb = const.tile([128, 18 * 32], BF16, tag="w")
    bsb = const.tile([128, 4], F32, tag="bias")

    # Step-0 act plane comes fully host-built (x copies + zeroed h + pads),
    # loaded in row chunks so group 0's matmuls start after ~19 rows land.
    nc.sync.dma_start(wsb[:, :], w_ap)
    nc.sync.dma_start(a0[:, 0 : 19 * WP], p0_ap[:, 0 : 19 * WP])
    for lo, hi in ((19, 51), (51, 83), (83, HP)):
        nc.sync.dma_start(a0[:, lo * WP : hi * WP], p0_ap[:, lo * WP : hi * WP])
    nc.sync.dma_start(bsb[:, :], b_ap)

    # --- one-time zero init (a1 h-region borders only; its x regions are
    # re-loaded host-padded every step, h interiors are scatter-written) ---
    ar1 = a1.rearrange("p (r w) -> p r w", r=HP)
    nc.gpsimd.memset(a1[32:64, 0:WP], 0.0)                  # pad row 0
    nc.gpsimd.memset(a1[96:128, 0:WP], 0.0)
    nc.gpsimd.memset(a1[32:64, (HP - 1) * WP :], 0.0)       # pad row 129
    nc.gpsimd.memset(a1[96:128, (HP - 1) * WP :], 0.0)
    nc.gpsimd.memset(ar1[32:64, :, 0:1], 0.0)               # col 0 (unread, keep finite)
    nc.gpsimd.memset(ar1[32:64, :, WP - 1 : WP], 0.0)       # center col 129
    nc.gpsimd.memset(ar1[96:128, :, 0:2], 0.0)              # shift cols 0-1
    nc.vector.memset(ctile[:, :], 0.0)

    def load_x(t):
        # x arrives host-padded to the full 130x130 plane: both the center
        # copy and the +1-shifted copy are single contiguous runs per
        # partition (the shift picks up its zero border from the host pad).
        nc.sync.dma_start(acts[t % 2][0:32, :], x_ap[t])
        nc.sync.dma_start(acts[t % 2][64:96, 1:PADN], x_ap[t, :, 0 : PADN - 1])

    from concourse.ap import AP as _AP

    def scatter_ap(a_nxt, pbase, q, j, coloff):
        """3D dst AP: partitions pbase..pbase+32, rows 64q+j+1 + 4k (k=0..16),
        cols coloff..coloff+128 of the 130x130 plane.  (Spatial tile j holds
        rows == j mod 4 of its quad, so the 16 staged rows form one stride-4
        run.)"""
        base = a_nxt[:, :]
        base_row = 64 * q + j + 1
        off = pbase * PADN + base_row * WP + coloff
        return _AP(
            base.tensor,
            base.offset + off,
            [[PADN, 32], [4 * WP, 16], [1, W]],
        )

    def pair_scatter_ap(a_nxt, pbase, pr, j, coloff):
        """Like scatter_ap but for one pair of groups: rows 32pr+j+1 + 4k,
        k=0..8."""
        base = a_nxt[:, :]
        off = pbase * PADN + (32 * pr + j + 1) * WP + coloff
        return _AP(
            base.tensor,
            base.offset + off,
            [[PADN, 32], [4 * WP, 8], [1, W]],
        )

    for t in range(n_steps):
        a_cur = acts[t % 2]
        a_nxt = acts[(t + 1) % 2]
        ar_cur = a_cur.rearrange("p (r w) -> p r w", r=HP)
        if t + 1 < n_steps:
            load_x(t + 1)

        hq = None
        pending = None
        for grp in range(GROUPS):
            if grp % 4 == 0:
                hq = tmp.tile(
                    [128, 4 * NT], BF16, tag="hq", name=f"hq{t}_{grp // 4}"
                )
            # Hybrid pass schedule, 18 slot-times per group (vs 24 all-full):
            # gate 2p: 3 full K=128 passes pairing (dy,0)+(dy,-1);
            # gate 2p+1: 3 full passes at col offset +1 pairing (dy,+1)+(dy,0);
            # leftovers -- 2p's (dy,+1) and 2p+1's (dy,-1) -- share 3 row-
            # split K=64 slots (center half / shifted half run concurrently
            # into their own PSUM banks via row tiling).
            zt = psum.tile([128, 4 * NT], F32, tag="z", name=f"z{t}_{grp}")
            for pg in range(2):
                ga, gb = 2 * pg, 2 * pg + 1
                for half, g in ((0, ga), (1, gb)):
                    for p3 in range(3):
                        dy = p3 - 1
                        blk = (pg * 9 + half * 3 + p3) * 32
                        for j in range(TPG):
                            r0 = 16 * grp + j + 1 + dy
                            nc.tensor.matmul(
                                zt[32 * j : 32 * j + 32, g * NT : (g + 1) * NT],
                                wsb[:, blk : blk + 32],
                                ar_cur[:, r0 : r0 + 13 : 4, 1 + half : W + 1 + half],
                                start=(p3 == 0),
                                stop=False,
                                skip_group_check=True,
                                tile_position=(0, 32 * j),
                            )
                for p3 in range(3):
                    dy = p3 - 1
                    blk = (pg * 9 + 6 + p3) * 32
                    for j in range(TPG):
                        r0 = 16 * grp + j + 1 + dy
                        rows = slice(r0, r0 + 13, 4)
                        nc.tensor.matmul(
                            zt[32 * j : 32 * j + 32, ga * NT : (ga + 1) * NT],
                            wsb[0:64, blk : blk + 32],
                            ar_cur[0:64, rows, 2 : W + 2],
                            start=False,
                            stop=(p3 == 2),
                            skip_group_check=True,
                            tile_position=(0, 32 * j),
                        )
                        nc.tensor.matmul(
                            zt[32 * j : 32 * j + 32, gb * NT : (gb + 1) * NT],
                            wsb[64:128, blk : blk + 32],
                            ar_cur[64:128, rows, 1 : W + 1],
                            start=False,
                            stop=(p3 == 2),
                            skip_group_check=True,
                            tile_position=(64, 32 * j),
                        )

            # ---- pointwise ----
            csl = ctile[:, grp * NT : (grp + 1) * NT]
            sio = tmp.tile([128, 3 * NT], BF16, tag="sio", name=f"sio{t}_{grp}")
            tg = tmp.tile([128, NT], BF16, tag="tg", name=f"tg{t}_{grp}")
            if zero_bias:
                nc.scalar.activation(sio[:, :], zt[:, 0 : 3 * NT], AF.Sigmoid)
                nc.scalar.activation(tg[:, :], zt[:, 3 * NT :], AF.Tanh)
            else:
                for g, sl in ((0, 0), (1, 1), (2, 2)):
                    nc.scalar.activation(
                        sio[:, sl * NT : (sl + 1) * NT],
                        zt[:, g * NT : (g + 1) * NT],
                        AF.Sigmoid,
                        bias=bsb[:, g : g + 1],
                    )
                nc.scalar.activation(
                    tg[:, :], zt[:, 3 * NT :], AF.Tanh, bias=bsb[:, 3:4]
                )
            si = sio[:, 0:NT]
            sf = sio[:, NT : 2 * NT]
            so = sio[:, 2 * NT : 3 * NT]

            t2 = tmp.tile([128, NT], F32, tag="t2", name=f"t2_{t}_{grp}")
            t3 = tmp.tile([128, NT], F32, tag="t3", name=f"t3_{t}_{grp}")
            nc.vector.tensor_mul(t3[:, :], sf, csl)
            nc.vector.tensor_mul(t2[:, :], si, tg[:, :])
            nc.vector.tensor_add(csl, t2[:, :], t3[:, :])

            # phase 2 (deferred one group): tanh(c), h, y, scatters.  Issuing
            # the previous group's tanh_c AFTER this group's gate acts keeps
            # the Scalar FIFO (and the next group's matmuls, which track the
            # sigmoid) from stalling on the DVE c-update chain.
            def flush(pgrp, phq, pcsl, pso):
                tcn = tmp.tile([128, NT], BF16, tag="tcn", name=f"tcn{t}_{pgrp}")
                nc.scalar.activation(tcn[:, :], pcsl, AF.Tanh)
                hsl = phq[:, (pgrp % 4) * NT : (pgrp % 4 + 1) * NT]
                nc.vector.tensor_mul(hsl, pso, tcn[:, :])
                if t == n_steps - 1:
                    nc.sync.dma_start(
                        y_ap[t, :, pgrp * NT : (pgrp + 1) * NT], hsl
                    )
                if pgrp % 2 == 1 and t < n_steps - 1:
                    # y per pair as well: halves the 512KB SBUF read burst
                    # that contended with the matmul rhs stream
                    bc0 = ((pgrp % 4) - 1) * NT
                    nc.sync.dma_start(
                        y_ap[t, :, (pgrp - 1) * NT : (pgrp + 1) * NT],
                        phq[:, bc0 : bc0 + 2 * NT],
                    )
                if pgrp % 2 == 1 and t + 1 < n_steps:
                    # scatter per PAIR of groups (half the burst of per-quad):
                    # the 9-DMA quad burst contended SBUF with the rhs stream
                    pr = pgrp // 2
                    bc = ((pgrp % 4) - 1) * NT
                    for j in range(TPG):
                        src = phq[32 * j : 32 * j + 32, bc : bc + 2 * NT]
                        nc.sync.dma_start(pair_scatter_ap(a_nxt, 32, pr, j, 1), src)
                        nc.sync.dma_start(pair_scatter_ap(a_nxt, 96, pr, j, 2), src)

            if pending is not None:
                flush(*pending)
            pending = (grp, hq, csl, so)
        flush(*pending)

    for p in reversed(ctx_pools):
        p.__exit__(None, None, None)


def build_program(n_steps=S, zero_bias=True):
    nc = bacc.Bacc(
        "TRN2",
        target_bir_lowering=False,
        debug=False,
        enable_asserts=False,
        num_devices=N_CORES,
    )
    x_d = nc.dram_tensor("x", [n_steps, CIN, PADN], BF16, kind="ExternalInput")
    w_d = nc.dram_tensor("w", [128, 18 * 32], BF16, kind="ExternalInput")
    b_d = nc.dram_tensor("bias", [128, 4], F32, kind="ExternalInput")
    y_d = nc.dram_tensor("y", [n_steps, 128, GROUPS * NT], BF16, kind="ExternalOutput")
    p0_d = nc.dram_tensor("plane0", [128, PADN], BF16, kind="ExternalInput")
    with tile.TileContext(nc) as tc:
        build_kernel(
            nc, tc, x_d.ap(), w_d.ap(), b_d.ap(), y_d.ap(), p0_d.ap(),
            n_steps, zero_bias,
        )
    nc.compile()
    return nc


def pack_weights(Wd):
    """Wd [128, 64, 3, 3] f32 -> lhsT blocks [128, 18*32] bf16.
    Per gate pair: 3 full blocks gate a (center kx=1, shift kx=0), 3 full
    blocks gate b (center kx=2, shift kx=1), 3 split blocks (rows 0-63 =
    gate a kx=2 via center; rows 64-127 = gate b kx=0 via shift)."""
    wp = np.zeros((128, 18, 32), np.float32)
    for pg in range(2):
        Wa = Wd[(2 * pg) * 32 : (2 * pg + 1) * 32]      # [32(m), 64, 3, 3]
        Wb_ = Wd[(2 * pg + 1) * 32 : (2 * pg + 2) * 32]
        for ky in range(3):
            blk = wp[:, pg * 9 + ky, :]
            blk[0:32, :] = Wa[:, 0:32, ky, 1].T
            blk[32:64, :] = Wa[:, 32:64, ky, 1].T
            blk[64:96, :] = Wa[:, 0:32, ky, 0].T
            blk[96:128, :] = Wa[:, 32:64, ky, 0].T
            blk = wp[:, pg * 9 + 3 + ky, :]
            blk[0:32, :] = Wb_[:, 0:32, ky, 2].T
            blk[32:64, :] = Wb_[:, 32:64, ky, 2].T
            blk[64:96, :] = Wb_[:, 0:32, ky, 1].T
            blk[96:128, :] = Wb_[:, 32:64, ky, 1].T
            blk = wp[:, pg * 9 + 6 + ky, :]
            blk[0:32, :] = Wa[:, 0:32, ky, 2].T
            blk[32:64, :] = Wa[:, 32:64, ky, 2].T
            blk[64:96, :] = Wb_[:, 0:32, ky, 0].T
            blk[96:128, :] = Wb_[:, 32:64, ky, 0].T
    return wp.reshape(128, 18 * 32).astype(ml_dtypes.bfloat16)


def pack_bias(bd):
    """bd [128] f32 -> [128, 4] f32 (partition p = 32*tile + ch)."""
    bp = np.zeros((128, 4), np.float32)
    for g in range(4):
        bp[:, g] = np.tile(bd[g * 32 : (g + 1) * 32], 4)
    return bp


def make_in_maps(x, Wf, bf, Wb, bb, n_steps=S):
    wpacks = [pack_weights(np.asarray(Wf, np.float32)),
              pack_weights(np.asarray(Wb, np.float32))]
    bpacks = [pack_bias(np.asarray(bf, np.float32)),
              pack_bias(np.asarray(bb, np.float32))]
    x = np.asarray(x, np.float32)
    in_maps = []
    for k in range(N_CORES):
        d, b = k // 4, k % 4
        xc = x[:n_steps, b] if d == 0 else x[::-1][:n_steps, b]
        xp = np.zeros((n_steps, CIN, HP, WP), ml_dtypes.bfloat16)
        xp[:, :, 1 : H + 1, 1 : W + 1] = xc
        p0 = np.zeros((128, HP, WP), ml_dtypes.bfloat16)
        p0[0:32] = xp[0]
        p0[64:96, :, 1:] = xp[0, :, :, :-1]
        in_maps.append(
            {
                "x": xp.reshape(n_steps, CIN, PADN),
                "w": wpacks[d],
                "bias": bpacks[d],
                "plane0": p0.reshape(128, PADN),
            }
        )
    return in_maps


_CACHED_NC = {}


def kernel(x, Wf, bf, Wb, bb):
    zero_bias = (not np.any(np.asarray(bf))) and (not np.any(np.asarray(bb)))
    nc = _CACHED_NC.get(zero_bias)
    if nc is None:
        nc = _CACHED_NC[zero_bias] = build_program(S, zero_bias)
    in_maps = make_in_maps(x, Wf, bf, Wb, bb)
    res = run_bass_kernel_spmd(nc, in_maps, core_ids=list(range(N_CORES)))
    out = np.empty((S, B, 2 * CO, H, W), np.float32)
    for k in range(N_CORES):
        d, b = k // 4, k % 4
        yk = np.asarray(res.results[k]["y"], dtype=np.float32)
        # [S, (j c), (q g r w)] -> [S, c, q, g, r, j, w]; row = 64q+16g+4r+j
        yk = yk.reshape(S, TPG, CO, 2, 4, 4, W).transpose(0, 2, 3, 4, 5, 1, 6)
        out[:, b, d * CO : (d + 1) * CO] = yk.reshape(S, CO, H, W)
    return out


if __name__ == "__main__":
    import jax

    jax.config.update("jax_platforms", "cpu")
    rng = np.random.default_rng(0)
    x = rng.standard_normal((S, B, CIN, H, W), np.float32)
    Wf = (rng.standard_normal((128, 64, 3, 3)) * 0.05).astype(np.float32)
    Wb = (rng.standard_normal((128, 64, 3, 3)) * 0.05).astype(np.float32)
    bf = np.zeros(128, np.float32)
    bb = np.zeros(128, np.float32)
    y = kernel(x, Wf, bf, Wb, bb)
    print("out", y.shape, y.dtype)
